# revision 41
# baseline (speedup 1.0000x reference)
"""Trainium2 Bass kernel for nn_Attention_24008867185039.

Reference computation (B=2, N=2048, DIM=1024, 16 heads x 64):
    q = x @ Wq ; k, v = split(x @ Wkv) ; per-head softmax(q k^T / sqrt(64)) v
    out = attn_out @ Wo + bo
(mask is all-ones per the problem spec, so masking is a no-op.)

Sharding (8 cores): data-parallel over batch (2) x tensor-parallel over 4
head-groups of 4 heads. Each core computes, for its (b, head-group):
  - Q^T, K^T projections [256, 2048] (d' on partitions -> ready for attention)
  - V projection [2048, 256] (+ a ones column per head so the PV matmul also
    produces softmax row-sums for free)
  - attention per head pair, unnormalized accumulation; per-column reciprocal
    normalization folded in before the output projection
  - partial output projection against its 256-row slice of Wo
Host sums the 4 Wo partials per batch and adds the bias.

Schedule: the PE matmul stream is at its hardware floor for bf16/f32r
(393216 columns = 163.8us @2.4GHz), so everything else is arranged to keep
the PE saturated:
  - the whole attention phase is one flat software-pipelined stream of 128
    (block, key-tile) steps: S(s+1) is emitted before PV(s) so the in-order
    PE never waits on an exp;
  - exp is split per (step, head)-half between the scalar engine (activation
    Exp -> bf16) and the DVE (one tensor_scalar computing a mean-centered
    Schraudolph exp directly in bf16 bit space: int16(s*184.665 + B) viewed
    as bf16), halving the per-engine exp stream so neither paces the PE;
  - projection chains and output-projection groups drip through the PE slack
    at a fixed step budget;
  - one PSUM pool of eight [128,512] banks serves both phases (tags: 4x S
    double-half-buffered, 3x O-accumulator ring, 1x drip); the prefix
    advances 8 concurrent chains per arriving x k-tile so the PE works
    through the x DMA stream;
  - the final block's normalization is sliced 128 queries at a time and its
    output-projection groups run double-buffered on the freed S banks so
    the tail drains at full PE rate.
"""

import sys

sys.path.insert(0, "/opt/trn_rl_repo")

import numpy as np

B, N, DIM, HEADS, DH = 2, 2048, 1024, 16, 64
HPG = 4                 # heads per core (head group)
DGRP = HPG * DH         # 256: per-core slice of the inner dim
NCORES = 8
KT = DIM // 128         # 8 contraction tiles for projections
NT = N // 128           # 16 sequence tiles of 128
NI = N // 512           # 4 query chunks of 512
MT = DGRP // 128        # 2 head-pair tiles per core

# Schraudolph exp in bf16 bit space, mean-centered for hardware
# round-to-nearest: i16 = rint(s*A + B); bf16 = bitcast(i16).
LOG2E = 1.4426950408889634
SCH_C = -0.0576
SCH_A = float(np.float32(128.0 * LOG2E))
SCH_B = float(np.float32(128.0 * (127.0 + SCH_C)))

_CACHE = {}

# schedule tunables (tuned against TimelineSim)
_CFG = {
    "chain_steps": {
        ("q", 0, 1): 2, ("q", 0, 2): 18, ("k", None, 0): 22,
        ("k", None, 1): 26, ("q", 0, 3): 34, ("k", None, 2): 38,
        ("k", None, 3): 42, ("q", 1, 0): 50, ("q", 1, 1): 66,
        ("q", 1, 2): 82, ("q", 1, 3): 98,
    },
    "op_offs": [3, 4, 5, 6, 7, 8, 9, 10, 12],
}


def _exp_on_dve(m, ig, j, half):
    # ~50/50 Act/DVE, alternating along j within every softmax row so the
    # Schraudolph sawtooth cancels in the softmax normalization.
    return (j + ig + half) % 2 == 0


def build_program(repeats=1):
    import concourse.mybir as mybir
    import concourse.tile as tile
    from concourse import bacc

    f32 = mybir.dt.float32

    nc = bacc.Bacc("TRN2", target_bir_lowering=False, debug=False,
                   num_devices=NCORES)

    bf16 = mybir.dt.bfloat16
    xt_d = nc.dram_tensor("xt", [DIM, N], bf16, kind="ExternalInput").ap()
    wq_d = nc.dram_tensor("wq", [DIM, DGRP], bf16, kind="ExternalInput").ap()
    wk_d = nc.dram_tensor("wk", [DIM, DGRP], bf16, kind="ExternalInput").ap()
    wv_d = nc.dram_tensor("wv", [DIM, DGRP], bf16, kind="ExternalInput").ap()
    wo_d = nc.dram_tensor("wo", [DGRP, DIM], f32, kind="ExternalInput").ap()
    # partials ship as bf16 (halves output DMA; host sums 4 partials in f32)
    part_d = nc.dram_tensor("part", [N, DIM], bf16,
                            kind="ExternalOutput").ap()

    with tile.TileContext(nc) as tc:
        for rep in range(repeats):
            _emit_body(nc, tc, xt_d, wq_d, wk_d, wv_d, wo_d, part_d,
                       tag=f"r{rep}")

    nc.compile()
    return nc


def _emit_body(nc, tc, xt_d, wq_d, wk_d, wv_d, wo_d, part_d, tag):
    import concourse.mybir as mybir
    from concourse.alu_op_type import AluOpType

    f32 = mybir.dt.float32
    bf16 = mybir.dt.bfloat16
    f32r = mybir.dt.float32r
    i16 = mybir.dt.int16
    Exp = mybir.ActivationFunctionType.Exp

    xt_t = xt_d.rearrange("(t p) n -> t p n", p=128)
    wq_t = wq_d.rearrange("(t p) d -> p t d", p=128)
    wk_t = wk_d.rearrange("(t p) d -> p t d", p=128)
    wv_t = wv_d.rearrange("(t p) d -> p t d", p=128)
    wo_t = wo_d.rearrange("(t p) d -> t p d", p=128)

    def r(ap):
        return ap.bitcast(f32r)

    with nc.allow_low_precision(reason="float32r/bf16/schraudolph intended"):
        with tc.tile_pool(name=f"persist{tag}", bufs=1) as pp, \
             tc.tile_pool(name=f"small{tag}", bufs=4) as sp, \
             tc.tile_pool(name=f"epool{tag}", bufs=8) as ep, \
             tc.tile_pool(name=f"osb{tag}", bufs=4) as op:

            # Persistent SBUF tensors
            qt = [[pp.tile([128, 512], f32r, name=f"qt{m}_{i}")
                   for i in range(NI)] for m in range(MT)]
            kt = [pp.tile([128, N], f32r, name=f"kt{m}") for m in range(MT)]
            # V with a ones column per head: [128, 4 heads x (64 d + 1)]
            vsb = [pp.tile([128, HPG * (DH + 1)], bf16, name=f"vsb{j}")
                   for j in range(NT)]
            wo_sb = [pp.tile([128, DIM], f32r, name=f"wo_sb{m}")
                     for m in range(MT)]
            ot0 = [pp.tile([128, 512], f32r, name=f"ot0_{i}")
                   for i in range(NI)]

            with tc.tile_pool(name=f"xw{tag}", bufs=1) as xp, \
                 tc.tile_pool(name=f"ps{tag}", bufs=1, space="PSUM") as pb:
                xt_sb = [xp.tile([128, N], bf16, name=f"xt{k}")
                         for k in range(KT)]
                wq_sb = xp.tile([128, KT, DGRP], bf16, name="wq_sb")
                wk_sb = xp.tile([128, KT, DGRP], bf16, name="wk_sb")
                wv_sb = xp.tile([128, KT, DGRP], bf16, name="wv_sb")
                # DMA order: x tiles are the scarce resource; weight pieces
                # arrive just before their first k-step use so the 565ns
                # SP setup cost per dma_start never delays the x stream.
                nc.sync.dma_start(out=wk_sb[:, 0:1, :], in_=wk_t[:, 0:1, :])
                nc.sync.dma_start(out=xt_sb[0][:, 0:512],
                                  in_=xt_t[0][:, 0:512])
                nc.sync.dma_start(out=xt_sb[0][:, 512:1024],
                                  in_=xt_t[0][:, 512:1024])
                nc.sync.dma_start(out=xt_sb[0][:, 1024:N],
                                  in_=xt_t[0][:, 1024:N])
                nc.sync.dma_start(out=wv_sb[:, 0:1, :], in_=wv_t[:, 0:1, :])
                nc.sync.dma_start(out=wk_sb[:, 1:KT, :],
                                  in_=wk_t[:, 1:KT, :])
                for k in range(1, KT):
                    nc.sync.dma_start(out=xt_sb[k][:], in_=xt_t[k])
                    if k == 2:
                        nc.sync.dma_start(out=wv_sb[:, 1:KT, :],
                                          in_=wv_t[:, 1:KT, :])
                nc.sync.dma_start(out=wq_sb[:], in_=wq_t)
                for m in range(MT):
                    nc.sync.dma_start(out=wo_sb[m][:], in_=r(wo_t[m]))

                def s_tile(name):
                    return pb.tile([128, 512], f32, name=name, tag="s_ps",
                                   bufs=4)

                def o_tile(name):
                    return pb.tile([128, 512], f32, name=name, tag="o_ps",
                                   bufs=3)

                def w_tile(name):
                    return pb.tile([128, 512], f32, name=name, tag="w_ps",
                                   bufs=1)

                # ---- Phase A: streamed prefix --------------------------
                # 8 concurrent accumulation chains (K m0 x4 on the S banks,
                # V0-2 on the O ring, V3 on the drip bank) advance per
                # arriving x k-tile so the PE works through the x DMA
                # stream; Q00 and V4..15 run back to back once x is loaded.
                k_ps = [s_tile(f"k_ps{i}") for i in range(NI)]
                v_ps = [o_tile(f"v_ps{j}") for j in range(3)] + \
                       [w_tile("v_ps3")]

                def v_finish(j, vp):
                    v3 = vsb[j].rearrange("p (h c) -> p h c", h=HPG)
                    nc.vector.tensor_copy(
                        out=v3[:, :, 0:DH],
                        in_=vp[:, 0:DGRP].rearrange("p (h c) -> p h c",
                                                    h=HPG))
                    nc.gpsimd.memset(v3[:, :, DH:DH + 1], 1.0)

                for k in range(KT):
                    st, sp_ = (k == 0), (k == KT - 1)
                    for i in range(NI):
                        nc.tensor.matmul(
                            k_ps[i][:], wk_sb[:, k, 0:128],
                            xt_sb[k][:, i * 512:(i + 1) * 512],
                            start=st, stop=sp_)
                        if sp_:
                            nc.scalar.copy(
                                out=kt[0][:, i * 512:(i + 1) * 512],
                                in_=k_ps[i][:])
                    for j in range(4):
                        nc.tensor.matmul(
                            v_ps[j][:, 0:DGRP],
                            xt_sb[k][:, j * 128:(j + 1) * 128],
                            wv_sb[:, k, :], start=st, stop=sp_)
                        if sp_:
                            v_finish(j, v_ps[j])
                q_ps = w_tile("q_ps")
                for k in range(KT):
                    nc.tensor.matmul(
                        q_ps[:], wq_sb[:, k, 0:128],
                        xt_sb[k][:, 0:512],
                        start=(k == 0), stop=(k == KT - 1))
                nc.scalar.copy(out=qt[0][0][:], in_=q_ps[:])
                for j in range(4, NT):
                    vp = o_tile("v_psx")
                    for k in range(KT):
                        nc.tensor.matmul(
                            vp[:, 0:DGRP],
                            xt_sb[k][:, j * 128:(j + 1) * 128],
                            wv_sb[:, k, :],
                            start=(k == 0), stop=(k == KT - 1))
                    v_finish(j, vp)

                # ---- Phase B: flat pipelined attention + drip ----------
                blocks = [(0, ig) for ig in range(NI)] + \
                         [(1, ig) for ig in range(NI)]
                steps = [(bi, j) for bi in range(len(blocks))
                         for j in range(NT)]
                NS = len(steps)

                o_acc = {}     # bi -> (o_psA, o_psB)
                e_tiles = {}   # s -> (eA_ap, eB_ap) as bf16
                s_tiles = {}   # s -> (s_psA, s_psB)

                def emit_S(s):
                    bi, j = steps[s]
                    m, ig = blocks[bi]
                    jsl = slice(j * 128, (j + 1) * 128)
                    sA = s_tile("s_psA")
                    sB = s_tile("s_psB")
                    nc.tensor.matmul(sA[:], kt[m][0:64, jsl],
                                     qt[m][ig][0:64, :],
                                     start=True, stop=True)
                    nc.tensor.matmul(sB[:], kt[m][64:128, jsl],
                                     qt[m][ig][64:128, :],
                                     start=True, stop=True)
                    s_tiles[s] = (sA, sB)

                def emit_exp(s):
                    bi, j = steps[s]
                    m, ig = blocks[bi]
                    sA, sB = s_tiles.pop(s)
                    res = []
                    for half, s_ps in ((0, sA), (1, sB)):
                        if _exp_on_dve(m, ig, j, half):
                            e = ep.tile([128, 512], i16, name="e_i16",
                                        tag="e")
                            nc.vector.tensor_scalar(
                                out=e[:], in0=s_ps[:],
                                scalar1=SCH_A, scalar2=SCH_B,
                                op0=AluOpType.mult, op1=AluOpType.add)
                            res.append(e[:].bitcast(bf16))
                        else:
                            e = ep.tile([128, 512], bf16, name="e_bf",
                                        tag="e")
                            nc.scalar.activation(out=e[:], in_=s_ps[:],
                                                 func=Exp)
                            res.append(e[:])
                    e_tiles[s] = res

                def emit_PV(s):
                    bi, j = steps[s]
                    m, _ = blocks[bi]
                    eA, eB = e_tiles.pop(s)
                    if j == 0:
                        o_acc[bi] = (o_tile("o_psA"), o_tile("o_psB"))
                    oA, oB = o_acc[bi]
                    hA, hB = 2 * m, 2 * m + 1
                    vA = slice(hA * (DH + 1), (hA + 1) * (DH + 1))
                    vB = slice(hB * (DH + 1), (hB + 1) * (DH + 1))
                    st, sp_ = (j == 0), (j == NT - 1)
                    nc.tensor.matmul(oA[0:DH + 1, :], vsb[j][:, vA], eA,
                                     start=st, stop=sp_)
                    nc.tensor.matmul(oB[0:DH + 1, :], vsb[j][:, vB], eB,
                                     start=st, stop=sp_)

                def emit_norm(bi, qsl, ot_dst, fast_free=True,
                              mulB_pool=False):
                    # normalize O^T columns in qsl by 1/rowsum. The A-side
                    # O bank gates the next block's PV(0) allocation, so it
                    # is first copied to SBUF by the scalar engine (frees
                    # the bank in ~0.7us) and normalized from there. At the
                    # tail the B-side mul runs on gpsimd so the two sides'
                    # chains overlap.
                    oA, oB = o_acc[bi]
                    n = qsl.stop - qsl.start
                    srcs = [(oA, qsl), (oB, qsl)]
                    for side in range(2):
                        if (side == 0 and fast_free) or \
                                (side == 1 and mulB_pool):
                            oc = sp.tile([DH + 1, 512], f32, name="oc",
                                         tag="oc")
                            nc.scalar.copy(out=oc[:, 0:n],
                                           in_=srcs[side][0][0:DH + 1, qsl])
                            srcs[side] = (oc, slice(0, n))
                    for side, (o_ps, ssl) in enumerate(srcs):
                        rr = sp.tile([1, 512], f32, name="rr", tag="rr")
                        nc.vector.reciprocal(rr[0:1, 0:n],
                                             o_ps[DH:DH + 1, ssl])
                        rb = sp.tile([DH, 512], f32, name="rb", tag="rb")
                        nc.gpsimd.partition_broadcast(rb[:, 0:n],
                                                      rr[0:1, 0:n])
                        eng = nc.gpsimd if (side == 1 and mulB_pool) \
                            else nc.vector
                        eng.tensor_mul(
                            out=ot_dst[side * DH:(side + 1) * DH, qsl],
                            in0=o_ps[0:DH, ssl], in1=rb[:, 0:n])

                # ---- drip work (runs in PE slack) ----------------------
                def proj_chain(w_sb, msl, dst, isl):
                    p_ps = w_tile("p_ps")
                    for k in range(KT):
                        nc.tensor.matmul(
                            p_ps[:], w_sb[:, k, msl], xt_sb[k][:, isl],
                            start=(k == 0), stop=(k == KT - 1))
                    nc.scalar.copy(out=dst, in_=p_ps[:])

                def q_chain(m, ich):
                    msl = slice(m * 128, (m + 1) * 128)
                    isl = slice(ich * 512, (ich + 1) * 512)
                    proj_chain(wq_sb, msl, qt[m][ich][:], isl)

                def k_chain(ich):
                    isl = slice(ich * 512, (ich + 1) * 512)
                    proj_chain(wk_sb, slice(128, 256), kt[1][:, isl], isl)

                out_sb_cur = [None]

                def outproj_group(ig, otB, gi, last, copy_eng=None):
                    jt, dch = gi // 2, gi % 2
                    if gi == 0:
                        out_sb_cur[0] = op.tile([128, 4096], bf16,
                                                name="out_sb",
                                                tag="out_sb", bufs=2)
                    out_sb = out_sb_cur[0]
                    lsl = slice(jt * 128, (jt + 1) * 128)
                    dsl = slice(dch * 512, (dch + 1) * 512)
                    # tail groups double-buffer on the freed S banks (and
                    # the O ring once its last norm reads retire)
                    if last:
                        o_ps = o_tile("t_po") if gi in (4, 5, 6) \
                            else s_tile("t_ps")
                    else:
                        o_ps = w_tile("out_ps")
                    pair = (ot0[ig], otB)
                    for m in range(MT):
                        nc.tensor.matmul(o_ps[:], pair[m][:, lsl],
                                         wo_sb[m][:, dsl],
                                         start=(m == 0),
                                         stop=(m == MT - 1))
                    csl = slice(jt * 1024 + dch * 512,
                                jt * 1024 + (dch + 1) * 512)
                    # alternate engines so neither builds an in-order
                    # backlog at block boundaries
                    if copy_eng is None:
                        copy_eng = "act" if gi % 2 == 0 else "dve"
                    if copy_eng == "act":
                        nc.scalar.copy(out=out_sb[:, csl], in_=o_ps[:])
                    else:
                        nc.vector.tensor_copy(out=out_sb[:, csl],
                                              in_=o_ps[:])


                def ship(ig, jt0, jt1):
                    out_sb = out_sb_cur[0]
                    part_v = part_d[ig * 512 + jt0 * 128:
                                    ig * 512 + jt1 * 128, :].rearrange(
                        "(jt p) d -> p jt d", p=128)
                    nc.sync.dma_start(
                        out=part_v,
                        in_=out_sb[:, jt0 * 1024:jt1 * 1024].rearrange(
                            "p (jt d) -> p jt d", jt=jt1 - jt0))

                # drip schedule: step index -> list of thunks.
                drip = {}

                def add_drip(step, fn):
                    drip.setdefault(step, []).append(fn)

                # remaining projections, placed inside earlier blocks:
                #  Q(0,1) in block0; Q(0,2) block1; Q(0,3) block2;
                #  K m1 x4 + Q(1,0) across blocks 1-3;
                #  Q(1,1..3) inside m1 blocks 4,5,6.
                chain_steps = _CFG["chain_steps"]
                for (kind, a, b), st in chain_steps.items():
                    if kind == "q":
                        add_drip(st, lambda a=a, b=b: q_chain(a, b))
                    else:
                        add_drip(st, lambda b=b: k_chain(b))

                # output projections for chunk ig drip inside block
                # (1, ig+1); the last chunk runs at the tail with sliced
                # normalization.
                otB_tiles = {}

                def schedule_outproj(ig, base_step):
                    # keep the last group 3+ steps clear of the next block
                    # boundary so its copy never stalls the boundary
                    otB = otB_tiles[ig]
                    offs = _CFG["op_offs"]
                    for gi in range(8):
                        off = min(offs[gi], NS - 3 - base_step)
                        add_drip(base_step + off,
                                 lambda ig=ig, otB=otB, gi=gi:
                                 outproj_group(ig, otB, gi, False))
                    add_drip(base_step + min(offs[8], NS - 2 - base_step),
                             lambda ig=ig: ship(ig, 0, 4))

                # ---- emit the flat stream ------------------------------
                emit_S(0)
                emit_exp(0)
                for s in range(NS):
                    if s + 1 < NS:
                        emit_S(s + 1)
                        emit_exp(s + 1)
                    emit_PV(s)
                    bi, j = steps[s]
                    m, ig = blocks[bi]
                    if j == NT - 1:
                        if bi < len(blocks) - 1:
                            if m == 0:
                                emit_norm(bi, slice(0, 512), ot0[ig][:])
                            else:
                                otB = op.tile([128, 512], f32r,
                                              name="otB", tag="otB",
                                              bufs=2)
                                otB_tiles[ig] = otB
                                emit_norm(bi, slice(0, 512), otB[:])
                                schedule_outproj(ig, s)
                        else:
                            # tail: sliced normalization + double-buffered
                            # output projection on the freed S banks
                            otB = op.tile([128, 512], f32r, name="otB",
                                          tag="otB", bufs=2)
                            otB_tiles[ig] = otB
                            # first 128 queries normalize alone for fast
                            # group(0) start; the remaining 384 as one set
                            for qsl in (slice(0, 128), slice(128, 512)):
                                emit_norm(bi, qsl, otB[:],
                                          fast_free=False)
                            for jt in range(4):
                                for dch in range(2):
                                    gi = 2 * jt + dch
                                    ce = "dve" if gi in (3, 5, 7) else "act"
                                    outproj_group(ig, otB, gi, True,
                                                  copy_eng=ce)
                                ship(ig, jt, jt + 1)
                    for fn in drip.pop(s, ()):
                        fn()
                for s in sorted(drip):
                    for fn in drip[s]:
                        fn()


def _get_nc():
    if "nc" not in _CACHE:
        _CACHE["nc"] = build_program()
    return _CACHE["nc"]


def make_in_maps(x, Wq, Wkv, Wo):
    import ml_dtypes

    bf16 = ml_dtypes.bfloat16
    scale = DH ** -0.5
    x = np.asarray(x, dtype=np.float32)
    Wq = np.asarray(Wq, dtype=np.float32)
    Wkv = np.asarray(Wkv, dtype=np.float32)
    Wo = np.asarray(Wo, dtype=np.float32)
    xt = [np.ascontiguousarray(x[b].T.astype(bf16)) for b in range(B)]
    in_maps = []
    for c in range(NCORES):
        b, hg = c // HPG, c % HPG
        sl = slice(hg * DGRP, (hg + 1) * DGRP)
        in_maps.append({
            "xt": xt[b],
            "wq": np.ascontiguousarray((Wq[:, sl] * scale).astype(bf16)),
            "wk": np.ascontiguousarray(
                Wkv[:, hg * DGRP:(hg + 1) * DGRP].astype(bf16)),
            "wv": np.ascontiguousarray(
                Wkv[:, DIM + hg * DGRP:DIM + (hg + 1) * DGRP].astype(bf16)),
            "wo": np.ascontiguousarray(Wo[sl, :]),
        })
    return in_maps


def combine_outputs(results, bo):
    out = np.zeros((B, N, DIM), dtype=np.float32)
    for c in range(NCORES):
        out[c // HPG] += results[c]["part"].astype(np.float32)
    out += np.asarray(bo, dtype=np.float32)
    return out


def kernel(x, mask, Wq, Wkv, Wo, bo):
    from concourse.bass_utils import run_bass_kernel_spmd

    nc = _get_nc()
    in_maps = make_in_maps(x, Wq, Wkv, Wo)
    res = run_bass_kernel_spmd(nc, in_maps, list(range(NCORES)))
    return combine_outputs(res.results, bo)


# revision 50
# speedup vs baseline: 1.0180x; 1.0180x over previous
"""Trainium2 Bass kernel for nn_Attention_24008867185039.

Reference computation (B=2, N=2048, DIM=1024, 16 heads x 64):
    q = x @ Wq ; k, v = split(x @ Wkv) ; per-head softmax(q k^T / sqrt(64)) v
    out = attn_out @ Wo + bo
(mask is all-ones per the problem spec, so masking is a no-op.)

Sharding (8 cores): data-parallel over batch (2) x tensor-parallel over 4
head-groups of 4 heads. Each core computes, for its (b, head-group):
  - Q^T, K^T projections [256, 2048] (d' on partitions -> ready for attention)
  - V projection [2048, 256] (+ a ones column per head so the PV matmul also
    produces softmax row-sums for free)
  - attention per head pair, unnormalized accumulation; per-column reciprocal
    normalization folded in before the output projection
  - partial output projection against its 256-row slice of Wo
Host sums the 4 Wo partials per batch and adds the bias.

Schedule: the PE matmul stream is at its hardware floor for bf16/f32r
(393216 columns = 163.8us @2.4GHz), so everything else is arranged to keep
the PE saturated:
  - the whole attention phase is one flat software-pipelined stream of 128
    (block, key-tile) steps: S(s+1) is emitted before PV(s) so the in-order
    PE never waits on an exp;
  - exp is split per (step, head)-half between the scalar engine (activation
    Exp -> bf16) and the DVE (one tensor_scalar computing a mean-centered
    Schraudolph exp directly in bf16 bit space: int16(s*184.665 + B) viewed
    as bf16), halving the per-engine exp stream so neither paces the PE;
  - projection chains and output-projection groups drip through the PE slack
    at a fixed step budget;
  - one PSUM pool of eight [128,512] banks serves both phases (tags: 4x S
    double-half-buffered, 3x O-accumulator ring, 1x drip); the prefix
    advances 8 concurrent chains per arriving x k-tile so the PE works
    through the x DMA stream;
  - the final block's normalization is sliced 128 queries at a time and its
    output-projection groups run double-buffered on the freed S banks so
    the tail drains at full PE rate.
"""

import sys

sys.path.insert(0, "/opt/trn_rl_repo")

import numpy as np

B, N, DIM, HEADS, DH = 2, 2048, 1024, 16, 64
HPG = 4                 # heads per core (head group)
DGRP = HPG * DH         # 256: per-core slice of the inner dim
NCORES = 8
KT = DIM // 128         # 8 contraction tiles for projections
NT = N // 128           # 16 sequence tiles of 128
NI = N // 512           # 4 query chunks of 512
MT = DGRP // 128        # 2 head-pair tiles per core

# Schraudolph exp in bf16 bit space, mean-centered for hardware
# round-to-nearest: i16 = rint(s*A + B); bf16 = bitcast(i16).
LOG2E = 1.4426950408889634
SCH_C = -0.0576
SCH_A = float(np.float32(128.0 * LOG2E))
SCH_B = float(np.float32(128.0 * (127.0 + SCH_C)))

_CACHE = {}

# schedule tunables (tuned against TimelineSim)
_CFG = {
    "chain_steps": {
        ("q", 0, 1): 1, ("q", 0, 2): 16, ("k", None, 0): 20,
        ("k", None, 1): 24, ("q", 0, 3): 32, ("k", None, 2): 36,
        ("k", None, 3): 40, ("q", 1, 0): 48, ("q", 1, 1): 64,
        ("q", 1, 2): 80, ("q", 1, 3): 96,
    },
    "op_offs": [3, 5, 7, 9, 11, 12, 13, 14, 15],
    # x1 before wk[1:] on the DMA queue + V chains before K chains per
    # k-group, so the k=1 V steps run while wk's tail transfers
    "x1_first": True,
    # tail shape: which group copies go to DVE, whether group 7 takes the
    # free drip bank, whether tail norm copies O to SBUF first
    "tail_dve_copies": (3, 5, 7),
    "tail_g7w": False,
    "tail_fastfree": False,
}


def _exp_on_dve(m, ig, j, half):
    # ~50/50 Act/DVE, alternating along j within every softmax row so the
    # Schraudolph sawtooth cancels in the softmax normalization.
    return (j + ig + half) % 2 == 0


def build_program(repeats=1):
    import concourse.mybir as mybir
    import concourse.tile as tile
    from concourse import bacc

    f32 = mybir.dt.float32

    nc = bacc.Bacc("TRN2", target_bir_lowering=False, debug=False,
                   num_devices=NCORES)

    bf16 = mybir.dt.bfloat16
    xt_d = nc.dram_tensor("xt", [DIM, N], bf16, kind="ExternalInput").ap()
    wq_d = nc.dram_tensor("wq", [DIM, DGRP], bf16, kind="ExternalInput").ap()
    wk_d = nc.dram_tensor("wk", [DIM, DGRP], bf16, kind="ExternalInput").ap()
    wv_d = nc.dram_tensor("wv", [DIM, DGRP], bf16, kind="ExternalInput").ap()
    wo_d = nc.dram_tensor("wo", [DGRP, DIM], f32, kind="ExternalInput").ap()
    # partials ship as bf16 (halves output DMA; host sums 4 partials in f32)
    part_d = nc.dram_tensor("part", [N, DIM], bf16,
                            kind="ExternalOutput").ap()

    with tile.TileContext(nc) as tc:
        for rep in range(repeats):
            _emit_body(nc, tc, xt_d, wq_d, wk_d, wv_d, wo_d, part_d,
                       tag=f"r{rep}")

    nc.compile()
    return nc


def _emit_body(nc, tc, xt_d, wq_d, wk_d, wv_d, wo_d, part_d, tag):
    import concourse.mybir as mybir
    from concourse.alu_op_type import AluOpType

    f32 = mybir.dt.float32
    bf16 = mybir.dt.bfloat16
    f32r = mybir.dt.float32r
    i16 = mybir.dt.int16
    Exp = mybir.ActivationFunctionType.Exp

    xt_t = xt_d.rearrange("(t p) n -> t p n", p=128)
    wq_t = wq_d.rearrange("(t p) d -> p t d", p=128)
    wk_t = wk_d.rearrange("(t p) d -> p t d", p=128)
    wv_t = wv_d.rearrange("(t p) d -> p t d", p=128)
    wo_t = wo_d.rearrange("(t p) d -> t p d", p=128)

    def r(ap):
        return ap.bitcast(f32r)

    with nc.allow_low_precision(reason="float32r/bf16/schraudolph intended"):
        with tc.tile_pool(name=f"persist{tag}", bufs=1) as pp, \
             tc.tile_pool(name=f"small{tag}", bufs=4) as sp, \
             tc.tile_pool(name=f"epool{tag}", bufs=8) as ep, \
             tc.tile_pool(name=f"osb{tag}", bufs=4) as op:

            # Persistent SBUF tensors
            qt = [[pp.tile([128, 512], f32r, name=f"qt{m}_{i}")
                   for i in range(NI)] for m in range(MT)]
            kt = [pp.tile([128, N], f32r, name=f"kt{m}") for m in range(MT)]
            # V with a ones column per head: [128, 4 heads x (64 d + 1)]
            vsb = [pp.tile([128, HPG * (DH + 1)], bf16, name=f"vsb{j}")
                   for j in range(NT)]
            wo_sb = [pp.tile([128, DIM], f32r, name=f"wo_sb{m}")
                     for m in range(MT)]
            ot0 = [pp.tile([128, 512], f32r, name=f"ot0_{i}")
                   for i in range(NI)]

            with tc.tile_pool(name=f"xw{tag}", bufs=1) as xp, \
                 tc.tile_pool(name=f"ps{tag}", bufs=1, space="PSUM") as pb:
                xt_sb = [xp.tile([128, N], bf16, name=f"xt{k}")
                         for k in range(KT)]
                wq_sb = xp.tile([128, KT, DGRP], bf16, name="wq_sb")
                wk_sb = xp.tile([128, KT, DGRP], bf16, name="wk_sb")
                wv_sb = xp.tile([128, KT, DGRP], bf16, name="wv_sb")
                # DMA order: x tiles are the scarce resource; weight pieces
                # arrive just before their first k-step use so the 565ns
                # SP setup cost per dma_start never delays the x stream.
                nc.sync.dma_start(out=wk_sb[:, 0:1, :], in_=wk_t[:, 0:1, :])
                nc.sync.dma_start(out=xt_sb[0][:, 0:512],
                                  in_=xt_t[0][:, 0:512])
                nc.sync.dma_start(out=xt_sb[0][:, 512:1024],
                                  in_=xt_t[0][:, 512:1024])
                nc.sync.dma_start(out=xt_sb[0][:, 1024:N],
                                  in_=xt_t[0][:, 1024:N])
                nc.sync.dma_start(out=wv_sb[:, 0:1, :], in_=wv_t[:, 0:1, :])
                if _CFG["x1_first"]:
                    nc.sync.dma_start(out=xt_sb[1][:], in_=xt_t[1])
                    nc.sync.dma_start(out=wk_sb[:, 1:KT, :],
                                      in_=wk_t[:, 1:KT, :])
                else:
                    nc.sync.dma_start(out=wk_sb[:, 1:KT, :],
                                      in_=wk_t[:, 1:KT, :])
                    nc.sync.dma_start(out=xt_sb[1][:], in_=xt_t[1])
                for k in range(2, KT):
                    nc.sync.dma_start(out=xt_sb[k][:], in_=xt_t[k])
                    if k == 2:
                        nc.sync.dma_start(out=wv_sb[:, 1:KT, :],
                                          in_=wv_t[:, 1:KT, :])
                nc.sync.dma_start(out=wq_sb[:], in_=wq_t)
                for m in range(MT):
                    nc.sync.dma_start(out=wo_sb[m][:], in_=r(wo_t[m]))

                def s_tile(name):
                    return pb.tile([128, 512], f32, name=name, tag="s_ps",
                                   bufs=4)

                def o_tile(name):
                    return pb.tile([128, 512], f32, name=name, tag="o_ps",
                                   bufs=3)

                def w_tile(name):
                    return pb.tile([128, 512], f32, name=name, tag="w_ps",
                                   bufs=1)

                # ---- Phase A: streamed prefix --------------------------
                # 12 concurrent accumulation chains advance per arriving x
                # k-tile so the PE works through the x DMA stream: K m0 x4
                # on the S banks, V0..7 paired two-per-bank (a V chain only
                # needs 256 PSUM columns) on the O ring + drip bank. Q00
                # and V8..15 run back to back once x is loaded.
                NVS = 8        # streamed V chains
                k_ps = [s_tile(f"k_ps{i}") for i in range(NI)]
                v_pair = [o_tile(f"v_pair{j}") for j in range(3)] + \
                         [w_tile("v_pair3")]

                def v_sl(vp, j):
                    c = (j % 2) * DGRP
                    return vp[:, c:c + DGRP]

                def v_finish(j, vp_sl):
                    v3 = vsb[j].rearrange("p (h c) -> p h c", h=HPG)
                    nc.vector.tensor_copy(
                        out=v3[:, :, 0:DH],
                        in_=vp_sl.rearrange("p (h c) -> p h c", h=HPG))
                    nc.gpsimd.memset(v3[:, :, DH:DH + 1], 1.0)

                for k in range(KT):
                    st, sp_ = (k == 0), (k == KT - 1)

                    def k_steps(k=k, st=st, sp_=sp_):
                        for i in range(NI):
                            nc.tensor.matmul(
                                k_ps[i][:], wk_sb[:, k, 0:128],
                                xt_sb[k][:, i * 512:(i + 1) * 512],
                                start=st, stop=sp_)
                            if sp_:
                                nc.scalar.copy(
                                    out=kt[0][:, i * 512:(i + 1) * 512],
                                    in_=k_ps[i][:])

                    def v_steps(k=k, st=st, sp_=sp_):
                        # paired chains: only the even chain starts/stops
                        # the bank's accumulation group. start=True zeroes
                        # the WHOLE bank's accumulate-bits, so the odd
                        # chain's first write (start=False, emitted right
                        # after) lands on pending-zero bytes and overwrites
                        # -- exactly a private start for its half.
                        for j in range(NVS):
                            odd = j % 2 == 1
                            nc.tensor.matmul(
                                v_sl(v_pair[j // 2], j),
                                xt_sb[k][:, j * 128:(j + 1) * 128],
                                wv_sb[:, k, :],
                                start=st and not odd, stop=sp_ and not odd,
                                skip_group_check=odd)
                            if sp_:
                                v_finish(j, v_sl(v_pair[j // 2], j))

                    # V first from k=1 on: V needs only wv0 + the x tile,
                    # so it runs while wk's tail is still transferring
                    if _CFG["x1_first"] and k >= 1:
                        v_steps()
                        k_steps()
                    else:
                        k_steps()
                        v_steps()
                q_ps = w_tile("q_ps")
                for k in range(KT):
                    nc.tensor.matmul(
                        q_ps[:], wq_sb[:, k, 0:128],
                        xt_sb[k][:, 0:512],
                        start=(k == 0), stop=(k == KT - 1))
                nc.scalar.copy(out=qt[0][0][:], in_=q_ps[:])
                for j in range(NVS, NT):
                    vp = o_tile("v_psx")
                    for k in range(KT):
                        nc.tensor.matmul(
                            vp[:, 0:DGRP],
                            xt_sb[k][:, j * 128:(j + 1) * 128],
                            wv_sb[:, k, :],
                            start=(k == 0), stop=(k == KT - 1))
                    v_finish(j, vp[:, 0:DGRP])

                # ---- Phase B: flat pipelined attention + drip ----------
                blocks = [(0, ig) for ig in range(NI)] + \
                         [(1, ig) for ig in range(NI)]
                steps = [(bi, j) for bi in range(len(blocks))
                         for j in range(NT)]
                NS = len(steps)

                o_acc = {}     # bi -> (o_psA, o_psB)
                e_tiles = {}   # s -> (eA_ap, eB_ap) as bf16
                s_tiles = {}   # s -> (s_psA, s_psB)

                def emit_S(s):
                    bi, j = steps[s]
                    m, ig = blocks[bi]
                    jsl = slice(j * 128, (j + 1) * 128)
                    sA = s_tile("s_psA")
                    sB = s_tile("s_psB")
                    nc.tensor.matmul(sA[:], kt[m][0:64, jsl],
                                     qt[m][ig][0:64, :],
                                     start=True, stop=True)
                    nc.tensor.matmul(sB[:], kt[m][64:128, jsl],
                                     qt[m][ig][64:128, :],
                                     start=True, stop=True)
                    s_tiles[s] = (sA, sB)

                def emit_exp(s):
                    bi, j = steps[s]
                    m, ig = blocks[bi]
                    sA, sB = s_tiles.pop(s)
                    res = []
                    for half, s_ps in ((0, sA), (1, sB)):
                        if _exp_on_dve(m, ig, j, half):
                            e = ep.tile([128, 512], i16, name="e_i16",
                                        tag="e")
                            nc.vector.tensor_scalar(
                                out=e[:], in0=s_ps[:],
                                scalar1=SCH_A, scalar2=SCH_B,
                                op0=AluOpType.mult, op1=AluOpType.add)
                            res.append(e[:].bitcast(bf16))
                        else:
                            e = ep.tile([128, 512], bf16, name="e_bf",
                                        tag="e")
                            nc.scalar.activation(out=e[:], in_=s_ps[:],
                                                 func=Exp)
                            res.append(e[:])
                    e_tiles[s] = res

                def emit_PV(s):
                    bi, j = steps[s]
                    m, _ = blocks[bi]
                    eA, eB = e_tiles.pop(s)
                    if j == 0:
                        o_acc[bi] = (o_tile("o_psA"), o_tile("o_psB"))
                    oA, oB = o_acc[bi]
                    hA, hB = 2 * m, 2 * m + 1
                    vA = slice(hA * (DH + 1), (hA + 1) * (DH + 1))
                    vB = slice(hB * (DH + 1), (hB + 1) * (DH + 1))
                    st, sp_ = (j == 0), (j == NT - 1)
                    nc.tensor.matmul(oA[0:DH + 1, :], vsb[j][:, vA], eA,
                                     start=st, stop=sp_)
                    nc.tensor.matmul(oB[0:DH + 1, :], vsb[j][:, vB], eB,
                                     start=st, stop=sp_)

                def emit_norm(bi, qsl, ot_dst, fast_free=True,
                              mulB_pool=False):
                    # normalize O^T columns in qsl by 1/rowsum. The A-side
                    # O bank gates the next block's PV(0) allocation, so it
                    # is first copied to SBUF by the scalar engine (frees
                    # the bank in ~0.7us) and normalized from there. At the
                    # tail the B-side mul runs on gpsimd so the two sides'
                    # chains overlap.
                    oA, oB = o_acc[bi]
                    n = qsl.stop - qsl.start
                    srcs = [(oA, qsl), (oB, qsl)]
                    for side in range(2):
                        if (side == 0 and fast_free) or \
                                (side == 1 and mulB_pool):
                            oc = sp.tile([DH + 1, 512], f32, name="oc",
                                         tag="oc")
                            nc.scalar.copy(out=oc[:, 0:n],
                                           in_=srcs[side][0][0:DH + 1, qsl])
                            srcs[side] = (oc, slice(0, n))
                    for side, (o_ps, ssl) in enumerate(srcs):
                        rr = sp.tile([1, 512], f32, name="rr", tag="rr")
                        nc.vector.reciprocal(rr[0:1, 0:n],
                                             o_ps[DH:DH + 1, ssl])
                        rb = sp.tile([DH, 512], f32, name="rb", tag="rb")
                        nc.gpsimd.partition_broadcast(rb[:, 0:n],
                                                      rr[0:1, 0:n])
                        eng = nc.gpsimd if (side == 1 and mulB_pool) \
                            else nc.vector
                        eng.tensor_mul(
                            out=ot_dst[side * DH:(side + 1) * DH, qsl],
                            in0=o_ps[0:DH, ssl], in1=rb[:, 0:n])

                # ---- drip work (runs in PE slack) ----------------------
                def proj_chain(w_sb, msl, dst, isl):
                    p_ps = w_tile("p_ps")
                    for k in range(KT):
                        nc.tensor.matmul(
                            p_ps[:], w_sb[:, k, msl], xt_sb[k][:, isl],
                            start=(k == 0), stop=(k == KT - 1))
                    nc.scalar.copy(out=dst, in_=p_ps[:])

                def q_chain(m, ich):
                    msl = slice(m * 128, (m + 1) * 128)
                    isl = slice(ich * 512, (ich + 1) * 512)
                    proj_chain(wq_sb, msl, qt[m][ich][:], isl)

                def k_chain(ich):
                    isl = slice(ich * 512, (ich + 1) * 512)
                    proj_chain(wk_sb, slice(128, 256), kt[1][:, isl], isl)

                out_sb_cur = [None]

                def outproj_group(ig, otB, gi, last, copy_eng=None):
                    jt, dch = gi // 2, gi % 2
                    if gi == 0:
                        out_sb_cur[0] = op.tile([128, 4096], bf16,
                                                name="out_sb",
                                                tag="out_sb", bufs=2)
                    out_sb = out_sb_cur[0]
                    lsl = slice(jt * 128, (jt + 1) * 128)
                    dsl = slice(dch * 512, (dch + 1) * 512)
                    # tail groups double-buffer on the freed S banks (and
                    # the O ring once its last norm reads retire)
                    if last:
                        if gi == 7 and _CFG["tail_g7w"]:
                            o_ps = w_tile("t_pw")
                        elif gi in (4, 5, 6):
                            o_ps = o_tile("t_po")
                        else:
                            o_ps = s_tile("t_ps")
                    else:
                        o_ps = w_tile("out_ps")
                    pair = (ot0[ig], otB)
                    for m in range(MT):
                        nc.tensor.matmul(o_ps[:], pair[m][:, lsl],
                                         wo_sb[m][:, dsl],
                                         start=(m == 0),
                                         stop=(m == MT - 1))
                    csl = slice(jt * 1024 + dch * 512,
                                jt * 1024 + (dch + 1) * 512)
                    # alternate engines so neither builds an in-order
                    # backlog at block boundaries
                    if copy_eng is None:
                        copy_eng = "act" if gi % 2 == 0 else "dve"
                    if copy_eng == "act":
                        nc.scalar.copy(out=out_sb[:, csl], in_=o_ps[:])
                    else:
                        nc.vector.tensor_copy(out=out_sb[:, csl],
                                              in_=o_ps[:])


                def ship(ig, jt0, jt1):
                    out_sb = out_sb_cur[0]
                    part_v = part_d[ig * 512 + jt0 * 128:
                                    ig * 512 + jt1 * 128, :].rearrange(
                        "(jt p) d -> p jt d", p=128)
                    nc.sync.dma_start(
                        out=part_v,
                        in_=out_sb[:, jt0 * 1024:jt1 * 1024].rearrange(
                            "p (jt d) -> p jt d", jt=jt1 - jt0))

                # drip schedule: step index -> list of thunks.
                drip = {}

                def add_drip(step, fn):
                    drip.setdefault(step, []).append(fn)

                # remaining projections, placed inside earlier blocks:
                #  Q(0,1) in block0; Q(0,2) block1; Q(0,3) block2;
                #  K m1 x4 + Q(1,0) across blocks 1-3;
                #  Q(1,1..3) inside m1 blocks 4,5,6.
                chain_steps = _CFG["chain_steps"]
                for (kind, a, b), st in chain_steps.items():
                    if kind == "q":
                        add_drip(st, lambda a=a, b=b: q_chain(a, b))
                    else:
                        add_drip(st, lambda b=b: k_chain(b))

                # output projections for chunk ig drip inside block
                # (1, ig+1); the last chunk runs at the tail with sliced
                # normalization.
                otB_tiles = {}

                def schedule_outproj(ig, base_step):
                    # keep the last group 3+ steps clear of the next block
                    # boundary so its copy never stalls the boundary
                    otB = otB_tiles[ig]
                    offs = _CFG["op_offs"]
                    for gi in range(8):
                        off = min(offs[gi], NS - 3 - base_step)
                        add_drip(base_step + off,
                                 lambda ig=ig, otB=otB, gi=gi:
                                 outproj_group(ig, otB, gi, False))
                    add_drip(base_step + min(offs[8], NS - 2 - base_step),
                             lambda ig=ig: ship(ig, 0, 4))

                # ---- emit the flat stream ------------------------------
                emit_S(0)
                emit_exp(0)
                for s in range(NS):
                    if s + 1 < NS:
                        emit_S(s + 1)
                        emit_exp(s + 1)
                    emit_PV(s)
                    bi, j = steps[s]
                    m, ig = blocks[bi]
                    if j == NT - 1:
                        if bi < len(blocks) - 1:
                            if m == 0:
                                emit_norm(bi, slice(0, 512), ot0[ig][:])
                            else:
                                otB = op.tile([128, 512], f32r,
                                              name="otB", tag="otB",
                                              bufs=2)
                                otB_tiles[ig] = otB
                                emit_norm(bi, slice(0, 512), otB[:])
                                schedule_outproj(ig, s)
                        else:
                            # tail: sliced normalization + double-buffered
                            # output projection on the freed S banks
                            otB = op.tile([128, 512], f32r, name="otB",
                                          tag="otB", bufs=2)
                            otB_tiles[ig] = otB
                            # first 128 queries normalize alone for fast
                            # group(0) start; the remaining 384 as one set
                            for qsl in (slice(0, 128), slice(128, 512)):
                                emit_norm(bi, qsl, otB[:],
                                          fast_free=_CFG["tail_fastfree"])
                            for jt in range(4):
                                for dch in range(2):
                                    gi = 2 * jt + dch
                                    ce = ("dve"
                                          if gi in _CFG["tail_dve_copies"]
                                          else "act")
                                    outproj_group(ig, otB, gi, True,
                                                  copy_eng=ce)
                                ship(ig, jt, jt + 1)
                    for fn in drip.pop(s, ()):
                        fn()
                for s in sorted(drip):
                    for fn in drip[s]:
                        fn()


def _get_nc():
    if "nc" not in _CACHE:
        _CACHE["nc"] = build_program()
    return _CACHE["nc"]


def make_in_maps(x, Wq, Wkv, Wo):
    import ml_dtypes

    bf16 = ml_dtypes.bfloat16
    scale = DH ** -0.5
    x = np.asarray(x, dtype=np.float32)
    Wq = np.asarray(Wq, dtype=np.float32)
    Wkv = np.asarray(Wkv, dtype=np.float32)
    Wo = np.asarray(Wo, dtype=np.float32)
    xt = [np.ascontiguousarray(x[b].T.astype(bf16)) for b in range(B)]
    in_maps = []
    for c in range(NCORES):
        b, hg = c // HPG, c % HPG
        sl = slice(hg * DGRP, (hg + 1) * DGRP)
        in_maps.append({
            "xt": xt[b],
            "wq": np.ascontiguousarray((Wq[:, sl] * scale).astype(bf16)),
            "wk": np.ascontiguousarray(
                Wkv[:, hg * DGRP:(hg + 1) * DGRP].astype(bf16)),
            "wv": np.ascontiguousarray(
                Wkv[:, DIM + hg * DGRP:DIM + (hg + 1) * DGRP].astype(bf16)),
            "wo": np.ascontiguousarray(Wo[sl, :]),
        })
    return in_maps


def combine_outputs(results, bo):
    out = np.zeros((B, N, DIM), dtype=np.float32)
    for c in range(NCORES):
        out[c // HPG] += results[c]["part"].astype(np.float32)
    out += np.asarray(bo, dtype=np.float32)
    return out


def kernel(x, mask, Wq, Wkv, Wo, bo):
    from concourse.bass_utils import run_bass_kernel_spmd

    nc = _get_nc()
    in_maps = make_in_maps(x, Wq, Wkv, Wo)
    res = run_bass_kernel_spmd(nc, in_maps, list(range(NCORES)))
    return combine_outputs(res.results, bo)


# revision 54
# speedup vs baseline: 1.0191x; 1.0011x over previous
"""Trainium2 Bass kernel for nn_Attention_24008867185039.

Reference computation (B=2, N=2048, DIM=1024, 16 heads x 64):
    q = x @ Wq ; k, v = split(x @ Wkv) ; per-head softmax(q k^T / sqrt(64)) v
    out = attn_out @ Wo + bo
(mask is all-ones per the problem spec, so masking is a no-op.)

Sharding (8 cores): data-parallel over batch (2) x tensor-parallel over 4
head-groups of 4 heads. Each core computes, for its (b, head-group):
  - Q^T, K^T projections [256, 2048] (d' on partitions -> ready for attention)
  - V projection [2048, 256] (+ a ones column per head so the PV matmul also
    produces softmax row-sums for free)
  - attention per head pair, unnormalized accumulation; per-column reciprocal
    normalization folded in before the output projection
  - partial output projection against its 256-row slice of Wo
Host sums the 4 Wo partials per batch and adds the bias.

Schedule: the PE matmul stream is at its hardware floor for bf16/f32r
(393216 columns = 163.8us @2.4GHz), so everything else is arranged to keep
the PE saturated:
  - the whole attention phase is one flat software-pipelined stream of 128
    (block, key-tile) steps: S(s+1) is emitted before PV(s) so the in-order
    PE never waits on an exp;
  - exp is split per (step, head)-half between the scalar engine (activation
    Exp -> bf16) and the DVE (one tensor_scalar computing a mean-centered
    Schraudolph exp directly in bf16 bit space: int16(s*184.665 + B) viewed
    as bf16), halving the per-engine exp stream so neither paces the PE;
  - projection chains and output-projection groups drip through the PE slack
    at a fixed step budget;
  - one PSUM pool of eight [128,512] banks serves both phases (tags: 4x S
    double-half-buffered, 3x O-accumulator ring, 1x drip); the prefix
    advances 8 concurrent chains per arriving x k-tile so the PE works
    through the x DMA stream;
  - the final block's normalization is sliced 128 queries at a time and its
    output-projection groups run double-buffered on the freed S banks so
    the tail drains at full PE rate.
"""

import sys

sys.path.insert(0, "/opt/trn_rl_repo")

import numpy as np

B, N, DIM, HEADS, DH = 2, 2048, 1024, 16, 64
HPG = 4                 # heads per core (head group)
DGRP = HPG * DH         # 256: per-core slice of the inner dim
NCORES = 8
KT = DIM // 128         # 8 contraction tiles for projections
NT = N // 128           # 16 sequence tiles of 128
NI = N // 512           # 4 query chunks of 512
MT = DGRP // 128        # 2 head-pair tiles per core

# Schraudolph exp in bf16 bit space, mean-centered for hardware
# round-to-nearest: i16 = rint(s*A + B); bf16 = bitcast(i16).
LOG2E = 1.4426950408889634
SCH_C = -0.0576
SCH_A = float(np.float32(128.0 * LOG2E))
SCH_B = float(np.float32(128.0 * (127.0 + SCH_C)))

_CACHE = {}

# schedule tunables (tuned against TimelineSim)
_CFG = {
    "chain_steps": {
        ("q", 0, 1): 1, ("q", 0, 2): 16, ("k", None, 0): 20,
        ("k", None, 1): 24, ("q", 0, 3): 32, ("k", None, 2): 36,
        ("k", None, 3): 40, ("q", 1, 0): 48, ("q", 1, 1): 64,
        ("q", 1, 2): 80, ("q", 1, 3): 96,
    },
    "op_offs": [3, 5, 7, 9, 11, 12, 13, 14, 15],
    # x1 before wk[1:] on the DMA queue + V chains before K chains per
    # k-group, so the k=1 V steps run while wk's tail transfers
    "x1_first": True,
    # tail shape: which group copies go to DVE, whether group 7 takes the
    # free drip bank, whether tail norm copies O to SBUF first
    "tail_dve_copies": (3, 5, 7),
    "tail_g7w": False,
    "tail_fastfree": False,
    # split the wv/wk weight tails into interleaved pieces so V/K k-steps
    # unblock together with their x tiles
    "w_split": True,
    # mid-stream B-side norm via Act copy + Pool mul (frees the DVE at
    # block boundaries)
    "norm_bpool_mid": False,
}


def _exp_on_dve(m, ig, j, half):
    # ~50/50 Act/DVE, alternating along j within every softmax row so the
    # Schraudolph sawtooth cancels in the softmax normalization.
    return (j + ig + half) % 2 == 0


def build_program(repeats=1):
    import concourse.mybir as mybir
    import concourse.tile as tile
    from concourse import bacc

    f32 = mybir.dt.float32

    nc = bacc.Bacc("TRN2", target_bir_lowering=False, debug=False,
                   num_devices=NCORES)

    bf16 = mybir.dt.bfloat16
    xt_d = nc.dram_tensor("xt", [DIM, N], bf16, kind="ExternalInput").ap()
    wq_d = nc.dram_tensor("wq", [DIM, DGRP], bf16, kind="ExternalInput").ap()
    wk_d = nc.dram_tensor("wk", [DIM, DGRP], bf16, kind="ExternalInput").ap()
    wv_d = nc.dram_tensor("wv", [DIM, DGRP], bf16, kind="ExternalInput").ap()
    wo_d = nc.dram_tensor("wo", [DGRP, DIM], f32, kind="ExternalInput").ap()
    # partials ship as bf16 (halves output DMA; host sums 4 partials in f32)
    part_d = nc.dram_tensor("part", [N, DIM], bf16,
                            kind="ExternalOutput").ap()

    with tile.TileContext(nc) as tc:
        for rep in range(repeats):
            _emit_body(nc, tc, xt_d, wq_d, wk_d, wv_d, wo_d, part_d,
                       tag=f"r{rep}")

    nc.compile()
    return nc


def _emit_body(nc, tc, xt_d, wq_d, wk_d, wv_d, wo_d, part_d, tag):
    import concourse.mybir as mybir
    from concourse.alu_op_type import AluOpType

    f32 = mybir.dt.float32
    bf16 = mybir.dt.bfloat16
    f32r = mybir.dt.float32r
    i16 = mybir.dt.int16
    Exp = mybir.ActivationFunctionType.Exp

    xt_t = xt_d.rearrange("(t p) n -> t p n", p=128)
    wq_t = wq_d.rearrange("(t p) d -> p t d", p=128)
    wk_t = wk_d.rearrange("(t p) d -> p t d", p=128)
    wv_t = wv_d.rearrange("(t p) d -> p t d", p=128)
    wo_t = wo_d.rearrange("(t p) d -> t p d", p=128)

    def r(ap):
        return ap.bitcast(f32r)

    with nc.allow_low_precision(reason="float32r/bf16/schraudolph intended"):
        with tc.tile_pool(name=f"persist{tag}", bufs=1) as pp, \
             tc.tile_pool(name=f"small{tag}", bufs=4) as sp, \
             tc.tile_pool(name=f"epool{tag}", bufs=8) as ep, \
             tc.tile_pool(name=f"osb{tag}", bufs=4) as op:

            # Persistent SBUF tensors
            qt = [[pp.tile([128, 512], f32r, name=f"qt{m}_{i}")
                   for i in range(NI)] for m in range(MT)]
            kt = [pp.tile([128, N], f32r, name=f"kt{m}") for m in range(MT)]
            # V with a ones column per head: [128, 4 heads x (64 d + 1)]
            vsb = [pp.tile([128, HPG * (DH + 1)], bf16, name=f"vsb{j}")
                   for j in range(NT)]
            wo_sb = [pp.tile([128, DIM], f32r, name=f"wo_sb{m}")
                     for m in range(MT)]
            ot0 = [pp.tile([128, 512], f32r, name=f"ot0_{i}")
                   for i in range(NI)]

            with tc.tile_pool(name=f"xw{tag}", bufs=1) as xp, \
                 tc.tile_pool(name=f"ps{tag}", bufs=1, space="PSUM") as pb:
                xt_sb = [xp.tile([128, N], bf16, name=f"xt{k}")
                         for k in range(KT)]
                wq_sb = xp.tile([128, KT, DGRP], bf16, name="wq_sb")
                wk_sb = xp.tile([128, KT, DGRP], bf16, name="wk_sb")
                wv_sb = xp.tile([128, KT, DGRP], bf16, name="wv_sb")
                # DMA order: x tiles are the scarce resource; weight pieces
                # arrive just before their first k-step use so the 565ns
                # SP setup cost per dma_start never delays the x stream.
                nc.sync.dma_start(out=wk_sb[:, 0:1, :], in_=wk_t[:, 0:1, :])
                nc.sync.dma_start(out=xt_sb[0][:, 0:512],
                                  in_=xt_t[0][:, 0:512])
                nc.sync.dma_start(out=xt_sb[0][:, 512:1024],
                                  in_=xt_t[0][:, 512:1024])
                nc.sync.dma_start(out=xt_sb[0][:, 1024:N],
                                  in_=xt_t[0][:, 1024:N])
                nc.sync.dma_start(out=wv_sb[:, 0:1, :], in_=wv_t[:, 0:1, :])
                if _CFG["w_split"]:
                    nc.sync.dma_start(out=wv_sb[:, 1:4, :],
                                      in_=wv_t[:, 1:4, :])
                    nc.sync.dma_start(out=xt_sb[1][:], in_=xt_t[1])
                    nc.sync.dma_start(out=wk_sb[:, 1:4, :],
                                      in_=wk_t[:, 1:4, :])
                    nc.sync.dma_start(out=xt_sb[2][:], in_=xt_t[2])
                    nc.sync.dma_start(out=wv_sb[:, 4:KT, :],
                                      in_=wv_t[:, 4:KT, :])
                    nc.sync.dma_start(out=xt_sb[3][:], in_=xt_t[3])
                    nc.sync.dma_start(out=wk_sb[:, 4:KT, :],
                                      in_=wk_t[:, 4:KT, :])
                    xrest = range(4, KT)
                elif _CFG["x1_first"]:
                    nc.sync.dma_start(out=xt_sb[1][:], in_=xt_t[1])
                    nc.sync.dma_start(out=wk_sb[:, 1:KT, :],
                                      in_=wk_t[:, 1:KT, :])
                    xrest = range(2, KT)
                else:
                    nc.sync.dma_start(out=wk_sb[:, 1:KT, :],
                                      in_=wk_t[:, 1:KT, :])
                    nc.sync.dma_start(out=xt_sb[1][:], in_=xt_t[1])
                    xrest = range(2, KT)
                for k in xrest:
                    nc.sync.dma_start(out=xt_sb[k][:], in_=xt_t[k])
                    if k == min(xrest) and not _CFG["w_split"]:
                        nc.sync.dma_start(out=wv_sb[:, 1:KT, :],
                                          in_=wv_t[:, 1:KT, :])
                nc.sync.dma_start(out=wq_sb[:], in_=wq_t)
                for m in range(MT):
                    nc.sync.dma_start(out=wo_sb[m][:], in_=r(wo_t[m]))

                def s_tile(name):
                    return pb.tile([128, 512], f32, name=name, tag="s_ps",
                                   bufs=4)

                def o_tile(name):
                    return pb.tile([128, 512], f32, name=name, tag="o_ps",
                                   bufs=3)

                def w_tile(name):
                    return pb.tile([128, 512], f32, name=name, tag="w_ps",
                                   bufs=1)

                # ---- Phase A: streamed prefix --------------------------
                # 12 concurrent accumulation chains advance per arriving x
                # k-tile so the PE works through the x DMA stream: K m0 x4
                # on the S banks, V0..7 paired two-per-bank (a V chain only
                # needs 256 PSUM columns) on the O ring + drip bank. Q00
                # and V8..15 run back to back once x is loaded.
                NVS = 8        # streamed V chains
                k_ps = [s_tile(f"k_ps{i}") for i in range(NI)]
                v_pair = [o_tile(f"v_pair{j}") for j in range(3)] + \
                         [w_tile("v_pair3")]

                def v_sl(vp, j):
                    c = (j % 2) * DGRP
                    return vp[:, c:c + DGRP]

                def v_finish(j, vp_sl):
                    v3 = vsb[j].rearrange("p (h c) -> p h c", h=HPG)
                    nc.vector.tensor_copy(
                        out=v3[:, :, 0:DH],
                        in_=vp_sl.rearrange("p (h c) -> p h c", h=HPG))
                    nc.gpsimd.memset(v3[:, :, DH:DH + 1], 1.0)

                for k in range(KT):
                    st, sp_ = (k == 0), (k == KT - 1)

                    def k_steps(k=k, st=st, sp_=sp_):
                        for i in range(NI):
                            nc.tensor.matmul(
                                k_ps[i][:], wk_sb[:, k, 0:128],
                                xt_sb[k][:, i * 512:(i + 1) * 512],
                                start=st, stop=sp_)
                            if sp_:
                                nc.scalar.copy(
                                    out=kt[0][:, i * 512:(i + 1) * 512],
                                    in_=k_ps[i][:])

                    def v_steps(k=k, st=st, sp_=sp_):
                        # paired chains: only the even chain starts/stops
                        # the bank's accumulation group. start=True zeroes
                        # the WHOLE bank's accumulate-bits, so the odd
                        # chain's first write (start=False, emitted right
                        # after) lands on pending-zero bytes and overwrites
                        # -- exactly a private start for its half.
                        for j in range(NVS):
                            odd = j % 2 == 1
                            nc.tensor.matmul(
                                v_sl(v_pair[j // 2], j),
                                xt_sb[k][:, j * 128:(j + 1) * 128],
                                wv_sb[:, k, :],
                                start=st and not odd, stop=sp_ and not odd,
                                skip_group_check=odd)
                            if sp_:
                                v_finish(j, v_sl(v_pair[j // 2], j))

                    # V first from k=1 on: V needs only wv0 + the x tile,
                    # so it runs while wk's tail is still transferring
                    if _CFG["x1_first"] and k >= 1:
                        v_steps()
                        k_steps()
                    else:
                        k_steps()
                        v_steps()
                q_ps = w_tile("q_ps")
                for k in range(KT):
                    nc.tensor.matmul(
                        q_ps[:], wq_sb[:, k, 0:128],
                        xt_sb[k][:, 0:512],
                        start=(k == 0), stop=(k == KT - 1))
                nc.scalar.copy(out=qt[0][0][:], in_=q_ps[:])
                for j in range(NVS, NT):
                    vp = o_tile("v_psx")
                    for k in range(KT):
                        nc.tensor.matmul(
                            vp[:, 0:DGRP],
                            xt_sb[k][:, j * 128:(j + 1) * 128],
                            wv_sb[:, k, :],
                            start=(k == 0), stop=(k == KT - 1))
                    v_finish(j, vp[:, 0:DGRP])

                # ---- Phase B: flat pipelined attention + drip ----------
                blocks = [(0, ig) for ig in range(NI)] + \
                         [(1, ig) for ig in range(NI)]
                steps = [(bi, j) for bi in range(len(blocks))
                         for j in range(NT)]
                NS = len(steps)

                o_acc = {}     # bi -> (o_psA, o_psB)
                e_tiles = {}   # s -> (eA_ap, eB_ap) as bf16
                s_tiles = {}   # s -> (s_psA, s_psB)

                def emit_S(s):
                    bi, j = steps[s]
                    m, ig = blocks[bi]
                    jsl = slice(j * 128, (j + 1) * 128)
                    sA = s_tile("s_psA")
                    sB = s_tile("s_psB")
                    nc.tensor.matmul(sA[:], kt[m][0:64, jsl],
                                     qt[m][ig][0:64, :],
                                     start=True, stop=True)
                    nc.tensor.matmul(sB[:], kt[m][64:128, jsl],
                                     qt[m][ig][64:128, :],
                                     start=True, stop=True)
                    s_tiles[s] = (sA, sB)

                def emit_exp(s):
                    bi, j = steps[s]
                    m, ig = blocks[bi]
                    sA, sB = s_tiles.pop(s)
                    res = []
                    for half, s_ps in ((0, sA), (1, sB)):
                        if _exp_on_dve(m, ig, j, half):
                            e = ep.tile([128, 512], i16, name="e_i16",
                                        tag="e")
                            nc.vector.tensor_scalar(
                                out=e[:], in0=s_ps[:],
                                scalar1=SCH_A, scalar2=SCH_B,
                                op0=AluOpType.mult, op1=AluOpType.add)
                            res.append(e[:].bitcast(bf16))
                        else:
                            e = ep.tile([128, 512], bf16, name="e_bf",
                                        tag="e")
                            nc.scalar.activation(out=e[:], in_=s_ps[:],
                                                 func=Exp)
                            res.append(e[:])
                    e_tiles[s] = res

                def emit_PV(s):
                    bi, j = steps[s]
                    m, _ = blocks[bi]
                    eA, eB = e_tiles.pop(s)
                    if j == 0:
                        o_acc[bi] = (o_tile("o_psA"), o_tile("o_psB"))
                    oA, oB = o_acc[bi]
                    hA, hB = 2 * m, 2 * m + 1
                    vA = slice(hA * (DH + 1), (hA + 1) * (DH + 1))
                    vB = slice(hB * (DH + 1), (hB + 1) * (DH + 1))
                    st, sp_ = (j == 0), (j == NT - 1)
                    nc.tensor.matmul(oA[0:DH + 1, :], vsb[j][:, vA], eA,
                                     start=st, stop=sp_)
                    nc.tensor.matmul(oB[0:DH + 1, :], vsb[j][:, vB], eB,
                                     start=st, stop=sp_)

                def emit_norm(bi, qsl, ot_dst, fast_free=True,
                              mulB_pool=False):
                    # normalize O^T columns in qsl by 1/rowsum. The A-side
                    # O bank gates the next block's PV(0) allocation, so it
                    # is first copied to SBUF by the scalar engine (frees
                    # the bank in ~0.7us) and normalized from there. At the
                    # tail the B-side mul runs on gpsimd so the two sides'
                    # chains overlap.
                    oA, oB = o_acc[bi]
                    n = qsl.stop - qsl.start
                    srcs = [(oA, qsl), (oB, qsl)]
                    for side in range(2):
                        if (side == 0 and fast_free) or \
                                (side == 1 and mulB_pool):
                            oc = sp.tile([DH + 1, 512], f32, name="oc",
                                         tag="oc")
                            nc.scalar.copy(out=oc[:, 0:n],
                                           in_=srcs[side][0][0:DH + 1, qsl])
                            srcs[side] = (oc, slice(0, n))
                    for side, (o_ps, ssl) in enumerate(srcs):
                        rr = sp.tile([1, 512], f32, name="rr", tag="rr")
                        nc.vector.reciprocal(rr[0:1, 0:n],
                                             o_ps[DH:DH + 1, ssl])
                        rb = sp.tile([DH, 512], f32, name="rb", tag="rb")
                        nc.gpsimd.partition_broadcast(rb[:, 0:n],
                                                      rr[0:1, 0:n])
                        eng = nc.gpsimd if (side == 1 and mulB_pool) \
                            else nc.vector
                        eng.tensor_mul(
                            out=ot_dst[side * DH:(side + 1) * DH, qsl],
                            in0=o_ps[0:DH, ssl], in1=rb[:, 0:n])

                # ---- drip work (runs in PE slack) ----------------------
                def proj_chain(w_sb, msl, dst, isl):
                    p_ps = w_tile("p_ps")
                    for k in range(KT):
                        nc.tensor.matmul(
                            p_ps[:], w_sb[:, k, msl], xt_sb[k][:, isl],
                            start=(k == 0), stop=(k == KT - 1))
                    nc.scalar.copy(out=dst, in_=p_ps[:])

                def q_chain(m, ich):
                    msl = slice(m * 128, (m + 1) * 128)
                    isl = slice(ich * 512, (ich + 1) * 512)
                    proj_chain(wq_sb, msl, qt[m][ich][:], isl)

                def k_chain(ich):
                    isl = slice(ich * 512, (ich + 1) * 512)
                    proj_chain(wk_sb, slice(128, 256), kt[1][:, isl], isl)

                out_sb_cur = [None]

                def outproj_group(ig, otB, gi, last, copy_eng=None):
                    jt, dch = gi // 2, gi % 2
                    if gi == 0:
                        out_sb_cur[0] = op.tile([128, 4096], bf16,
                                                name="out_sb",
                                                tag="out_sb", bufs=2)
                    out_sb = out_sb_cur[0]
                    lsl = slice(jt * 128, (jt + 1) * 128)
                    dsl = slice(dch * 512, (dch + 1) * 512)
                    # tail groups double-buffer on the freed S banks (and
                    # the O ring once its last norm reads retire)
                    if last:
                        if gi == 7 and _CFG["tail_g7w"]:
                            o_ps = w_tile("t_pw")
                        elif gi in (4, 5, 6):
                            o_ps = o_tile("t_po")
                        else:
                            o_ps = s_tile("t_ps")
                    else:
                        o_ps = w_tile("out_ps")
                    pair = (ot0[ig], otB)
                    for m in range(MT):
                        nc.tensor.matmul(o_ps[:], pair[m][:, lsl],
                                         wo_sb[m][:, dsl],
                                         start=(m == 0),
                                         stop=(m == MT - 1))
                    csl = slice(jt * 1024 + dch * 512,
                                jt * 1024 + (dch + 1) * 512)
                    # alternate engines so neither builds an in-order
                    # backlog at block boundaries
                    if copy_eng is None:
                        copy_eng = "act" if gi % 2 == 0 else "dve"
                    if copy_eng == "act":
                        nc.scalar.copy(out=out_sb[:, csl], in_=o_ps[:])
                    else:
                        nc.vector.tensor_copy(out=out_sb[:, csl],
                                              in_=o_ps[:])


                def ship(ig, jt0, jt1):
                    out_sb = out_sb_cur[0]
                    part_v = part_d[ig * 512 + jt0 * 128:
                                    ig * 512 + jt1 * 128, :].rearrange(
                        "(jt p) d -> p jt d", p=128)
                    nc.sync.dma_start(
                        out=part_v,
                        in_=out_sb[:, jt0 * 1024:jt1 * 1024].rearrange(
                            "p (jt d) -> p jt d", jt=jt1 - jt0))

                # drip schedule: step index -> list of thunks.
                drip = {}

                def add_drip(step, fn):
                    drip.setdefault(step, []).append(fn)

                # remaining projections, placed inside earlier blocks:
                #  Q(0,1) in block0; Q(0,2) block1; Q(0,3) block2;
                #  K m1 x4 + Q(1,0) across blocks 1-3;
                #  Q(1,1..3) inside m1 blocks 4,5,6.
                chain_steps = _CFG["chain_steps"]
                for (kind, a, b), st in chain_steps.items():
                    if kind == "q":
                        add_drip(st, lambda a=a, b=b: q_chain(a, b))
                    else:
                        add_drip(st, lambda b=b: k_chain(b))

                # output projections for chunk ig drip inside block
                # (1, ig+1); the last chunk runs at the tail with sliced
                # normalization.
                otB_tiles = {}

                def schedule_outproj(ig, base_step):
                    # keep the last group 3+ steps clear of the next block
                    # boundary so its copy never stalls the boundary
                    otB = otB_tiles[ig]
                    offs = _CFG["op_offs"]
                    for gi in range(8):
                        off = min(offs[gi], NS - 3 - base_step)
                        add_drip(base_step + off,
                                 lambda ig=ig, otB=otB, gi=gi:
                                 outproj_group(ig, otB, gi, False))
                    add_drip(base_step + min(offs[8], NS - 2 - base_step),
                             lambda ig=ig: ship(ig, 0, 4))

                # ---- emit the flat stream ------------------------------
                emit_S(0)
                emit_exp(0)
                for s in range(NS):
                    if s + 1 < NS:
                        emit_S(s + 1)
                        emit_exp(s + 1)
                    emit_PV(s)
                    bi, j = steps[s]
                    m, ig = blocks[bi]
                    if j == NT - 1:
                        if bi < len(blocks) - 1:
                            bp = _CFG["norm_bpool_mid"]
                            if m == 0:
                                emit_norm(bi, slice(0, 512), ot0[ig][:],
                                          mulB_pool=bp)
                            else:
                                otB = op.tile([128, 512], f32r,
                                              name="otB", tag="otB",
                                              bufs=2)
                                otB_tiles[ig] = otB
                                emit_norm(bi, slice(0, 512), otB[:],
                                          mulB_pool=bp)
                                schedule_outproj(ig, s)
                        else:
                            # tail: sliced normalization + double-buffered
                            # output projection on the freed S banks
                            otB = op.tile([128, 512], f32r, name="otB",
                                          tag="otB", bufs=2)
                            otB_tiles[ig] = otB
                            # first 128 queries normalize alone for fast
                            # group(0) start; the remaining 384 as one set
                            for qsl in (slice(0, 128), slice(128, 512)):
                                emit_norm(bi, qsl, otB[:],
                                          fast_free=_CFG["tail_fastfree"])
                            for jt in range(4):
                                for dch in range(2):
                                    gi = 2 * jt + dch
                                    ce = ("dve"
                                          if gi in _CFG["tail_dve_copies"]
                                          else "act")
                                    outproj_group(ig, otB, gi, True,
                                                  copy_eng=ce)
                                ship(ig, jt, jt + 1)
                    for fn in drip.pop(s, ()):
                        fn()
                for s in sorted(drip):
                    for fn in drip[s]:
                        fn()


def _get_nc():
    if "nc" not in _CACHE:
        _CACHE["nc"] = build_program()
    return _CACHE["nc"]


def make_in_maps(x, Wq, Wkv, Wo):
    import ml_dtypes

    bf16 = ml_dtypes.bfloat16
    scale = DH ** -0.5
    x = np.asarray(x, dtype=np.float32)
    Wq = np.asarray(Wq, dtype=np.float32)
    Wkv = np.asarray(Wkv, dtype=np.float32)
    Wo = np.asarray(Wo, dtype=np.float32)
    xt = [np.ascontiguousarray(x[b].T.astype(bf16)) for b in range(B)]
    in_maps = []
    for c in range(NCORES):
        b, hg = c // HPG, c % HPG
        sl = slice(hg * DGRP, (hg + 1) * DGRP)
        in_maps.append({
            "xt": xt[b],
            "wq": np.ascontiguousarray((Wq[:, sl] * scale).astype(bf16)),
            "wk": np.ascontiguousarray(
                Wkv[:, hg * DGRP:(hg + 1) * DGRP].astype(bf16)),
            "wv": np.ascontiguousarray(
                Wkv[:, DIM + hg * DGRP:DIM + (hg + 1) * DGRP].astype(bf16)),
            "wo": np.ascontiguousarray(Wo[sl, :]),
        })
    return in_maps


def combine_outputs(results, bo):
    out = np.zeros((B, N, DIM), dtype=np.float32)
    for c in range(NCORES):
        out[c // HPG] += results[c]["part"].astype(np.float32)
    out += np.asarray(bo, dtype=np.float32)
    return out


def kernel(x, mask, Wq, Wkv, Wo, bo):
    from concourse.bass_utils import run_bass_kernel_spmd

    nc = _get_nc()
    in_maps = make_in_maps(x, Wq, Wkv, Wo)
    res = run_bass_kernel_spmd(nc, in_maps, list(range(NCORES)))
    return combine_outputs(res.results, bo)
